# revision 3
# baseline (speedup 1.0000x reference)
"""DeepSpeedMLP Trainium2 kernel.

Computation (per reference):
    x   = input + bias + residual
    h   = LayerNorm(x) * ln_w + attn_nb          (ln_w == ones)
    h1  = relu(h @ inter_w + inter_b)
    out = h1 @ output_w + output_b + x

Sharding: pure data parallel over tokens. B*S = 8192 tokens split across
8 cores (1024 tokens each); weights replicated (cast to bf16 host-side).

Host-side folds (exact, by linearity): ln_w into inter_w rows;
attn_nb @ inter_w + inter_b into the fc1 bias; input+residual packed as
one bf16 [TOK, 2, H] tensor (halves input DMA; LN runs in bf16 anyway).

Per-core dataflow (matmuls in bf16 with fp32 PSUM accumulation). The
core's tokens are processed in two 512-token halves, each running
LayerNorm -> fc1 -> fc2 end-to-end; the next half's LayerNorm (DVE) runs
under the previous half's matmuls (PE):
  phase 1 (per 128-token block): x = (in+bias) + res (one bf16 DVE add,
           bias pre-added host-side); x + output_b -> bf16 DRAM scratch (re-read in fc2 for
           the residual+output-bias add); LayerNorm stats via
           bn_stats/bn_aggr; normalized bf16 rows PE-transposed into
           xT [H, tok-half], 8 transposes batched per PSUM bank and
           drained with one wide ACT copy.
  fc1:     h1T[I-chunk, tok-half] = relu(W1-chunk.T @ xT + b1') --
           weight-stationary matmuls accumulating over H; ACT applies
           bias+relu+bf16-cast straight out of PSUM.
  fc2:     token-stationary ("flip"): out[tok-block, H-quarter]
           accumulated over I with lhsT = h1T 128-token blocks and
           rhs = w2 rows; no output transposes. Drain = one DVE add of
           the x+output_b scratch, then a wide-row DMA out.

Queue split: the SP DMA queue carries only the load streams (inputs,
w1/w2 weights, xpb scratch); drain-side DMAs (residual-row prefetch,
output stores) ride the Activation HWDGE queue, so a store waiting on
its producer never head-of-line blocks the next tile's weight stream.
fc1/fc2 accumulators share a 7-bank PSUM rotation (+1 bank for batched
transposes); emission order p1(A) fc1(A) p1(B) fc2(A) fc1(B) fc2(B)
hides both LayerNorms under matmuls.
"""

import numpy as np
import ml_dtypes

_B, _S, _H, _I = 4, 2048, 2048, 8192
_NCORES = 8
_LN_EPS = 1e-5

_CACHE = {}


def _build(TOK, H, I, repeat=1):
    """Build + compile the per-core Bass kernel. Returns the compiled Bacc."""
    from contextlib import ExitStack

    import concourse.bass as bass
    import concourse.mybir as mybir
    import concourse.tile as tile
    from concourse import bacc
    from concourse.masks import make_identity

    f32 = mybir.dt.float32
    bf16 = mybir.dt.bfloat16
    Alu = mybir.AluOpType
    Act = mybir.ActivationFunctionType

    P = 128
    Hk = H // P          # H chunks (fc1 contraction / xT partition tiles)
    Im = I // P          # I chunks (fc1 output tiles / fc2 contraction)
    TB = TOK // P        # token blocks
    NH = TOK // 2        # tokens per half == matmul free dim, <= 512
    TBH = TB // 2        # token blocks per half
    NQ = 512             # fc2 output column chunk
    HQ = H // NQ
    assert NH <= 512 and TB % 2 == 0
    SG = max(H // 512, 1)  # bn_stats subgroups

    nc = bacc.Bacc("TRN2", target_bir_lowering=False, debug=False)

    # input and residual packed host-side as bf16 [TOK, 2, H]: one DMA per
    # token block and half the input traffic
    xr_in = nc.dram_tensor("xr_in", [TOK, 2, H], bf16, kind="ExternalInput")
    w1 = nc.dram_tensor("w1", [Im, P, Hk, P], bf16, kind="ExternalInput")
    b1 = nc.dram_tensor("b1", [P, Im], f32, kind="ExternalInput")
    w2 = nc.dram_tensor("w2", [Im, P, H], bf16, kind="ExternalInput")
    b2_row = nc.dram_tensor("b2_row", [H], bf16, kind="ExternalInput")
    out_d = nc.dram_tensor("out", [TOK, H], f32, kind="ExternalOutput")
    xpb_d = nc.dram_tensor("xpb_scratch", [TOK, H], bf16)

    def brd(vec_ap):  # broadcast a [H] dram vector across 128 partitions
        return bass.AP(
            tensor=vec_ap.tensor, offset=vec_ap.offset, ap=[[0, P], *vec_ap.ap]
        )

    with tile.TileContext(nc) as tc:
        with ExitStack() as st:
            consts = st.enter_context(tc.tile_pool(name="consts", bufs=1))
            ident_bf = consts.tile([P, P], bf16)
            make_identity(nc, ident_bf)
            b1_sb = consts.tile([P, Im], f32)
            nc.scalar.dma_start(out=b1_sb, in_=b1[:])
            eps_sb = consts.tile([P, 1], f32)
            nc.vector.memset(eps_sb, _LN_EPS)
            # b2_rep DMA is deferred until after the first input block's
            # load so it doesn't delay the LN pipeline
            b2_rep = consts.tile([P, H], bf16)

            big = st.enter_context(tc.tile_pool(name="big", bufs=1))
            io = st.enter_context(tc.tile_pool(name="io", bufs=3))
            lnp = st.enter_context(tc.tile_pool(name="lnp", bufs=2))
            w1p = st.enter_context(tc.tile_pool(name="w1p", bufs=3))
            w2p = st.enter_context(tc.tile_pool(name="w2p", bufs=3))
            xqp = st.enter_context(tc.tile_pool(name="xqp", bufs=3))
            outp = st.enter_context(tc.tile_pool(name="outp", bufs=6))
            PS = bass.MemorySpace.PSUM
            # PSUM budget: 8 banks = ps_tr 1 + ps_acc 7. fc1 and fc2
            # accumulators share one 7-deep rotation (same tag), so a new
            # accumulator group always lands on a long-drained bank.
            ps_tr = st.enter_context(tc.tile_pool(name="ps_tr", bufs=1, space=PS))
            ps_acc = st.enter_context(tc.tile_pool(name="ps_acc", bufs=7, space=PS))

            # Queue split: SP (nc.sync) carries ONLY loads; stores (xpb
            # scratch, out rows) go on the Activation HWDGE queue
            # (nc.scalar.dma_start). A store waits on its producer at the
            # queue head, so mixing them with loads head-of-line blocks the
            # next tile's weight stream at every drain point.
            xTs, h1Ts = {}, {}

            def phase1(half):
                xT = big.tile([P, Hk, NH], bf16, tag="xT", bufs=2, name="xT")
                xTs[half] = xT
                for jh in range(TBH):
                    j = half * TBH + jh
                    xrt = io.tile([P, 2, H], bf16, tag="io")
                    nc.sync.dma_start(out=xrt, in_=xr_in[j * P:(j + 1) * P])
                    if half == 0 and jh == 0:
                        nc.scalar.dma_start(out=b2_rep, in_=brd(b2_row[:]))

                    # LN pipeline in bf16: 2x DVE throughput on the add,
                    # stats and normalize; ~0.3% extra error, well in budget.
                    # bias is pre-added into xr[:,0,:] host-side.
                    xt = lnp.tile([P, H], bf16, tag="x")
                    nc.vector.tensor_add(
                        out=xt, in0=xrt[:, 0, :], in1=xrt[:, 1, :]
                    )

                    stats = lnp.tile([P, SG, 6], f32, tag="stats")
                    xg = xt.rearrange("p (n f) -> p n f", n=SG)
                    for g in range(SG):
                        nc.vector.bn_stats(out=stats[:, g, :], in_=xg[:, g, :])
                    mv = lnp.tile([P, 2], f32, tag="mv")
                    nc.vector.bn_aggr(out=mv, in_=stats)
                    rstd = lnp.tile([P, 1], f32, tag="rstd")
                    nc.scalar.activation(
                        out=rstd, in_=mv[:, 1:2], func=Act.Sqrt, bias=eps_sb
                    )
                    nc.vector.reciprocal(out=rstd, in_=rstd)

                    hf = lnp.tile([P, H], bf16, tag="hf")
                    nc.vector.tensor_scalar(
                        out=hf,
                        in0=xt,
                        scalar1=mv[:, 0:1],
                        scalar2=rstd,
                        op0=Alu.subtract,
                        op1=Alu.mult,
                    )
                    # x + output_b, saved (bf16) for the fc2 drain; emitted
                    # after the normalize, stored on the SP queue
                    xpbt = lnp.tile([P, H], bf16, tag="xpb")
                    nc.vector.tensor_add(out=xpbt, in0=xt, in1=b2_rep)
                    # transpose 8 H-chunks into one PSUM bank, then drain
                    # them with a single wide ACT copy (attn_nb is folded
                    # into b1 host-side, so the drain is a plain copy)
                    for kg in range(Hk // 8):
                        pt = ps_tr.tile([P, 8, P], bf16, tag="pt")
                        for k8 in range(8):
                            nc.tensor.transpose(
                                out=pt[:, k8, :],
                                in_=hf[
                                    :, (kg * 8 + k8) * P:(kg * 8 + k8 + 1) * P
                                ],
                                identity=ident_bf,
                            )
                        nc.scalar.activation(
                            out=xT[:, kg * 8:(kg + 1) * 8, jh * P:(jh + 1) * P],
                            in_=pt,
                            func=Act.Identity,
                        )
                    nc.sync.dma_start(
                        out=xpb_d[j * P:(j + 1) * P, :], in_=xpbt
                    )

            def fc1(half):
                xT = xTs[half]
                h1T = big.tile([P, Im, NH], bf16, tag="h1T", name="h1T")
                h1Ts[half] = h1T
                for m in range(Im):
                    w1t = w1p.tile([P, Hk, P], bf16, tag="w1")
                    nc.sync.dma_start(out=w1t, in_=w1[m])
                    ps = ps_acc.tile([P, NH], f32, tag="acc", name="mm1")
                    for k in range(Hk):
                        nc.tensor.matmul(
                            ps,
                            lhsT=w1t[:, k, :],
                            rhs=xT[:, k, :],
                            start=(k == 0),
                            stop=(k == Hk - 1),
                        )
                    nc.scalar.activation(
                        out=h1T[:, m, :],
                        in_=ps,
                        func=Act.Relu,
                        bias=b1_sb[:, m:m + 1],
                        scale=1.0,
                    )

            # DMA packing: HWDGE costs ~625ns of setup per DMA regardless of
            # size, so w2 rows are loaded 4 k2-chunks per DMA and the
            # residual rows 4 token-blocks per DMA.
            w2_r = w2[:].rearrange("a p h -> p a h")
            xpb_r = xpb_d[:].rearrange("(j p) h -> p j h", p=P)

            def fc2(half):
                h1T = h1Ts[half]
                for hq in range(HQ):
                    ho = hq * NQ
                    ps2 = [
                        ps_acc.tile([P, NQ], f32, tag="acc", name=f"ps2_{tb}")
                        for tb in range(TBH)
                    ]
                    # prefetch the x+output_b rows for this hq's drains
                    # (ACT queue: keeps the SP queue a pure weight stream)
                    xqt = xqp.tile([P, TBH, NQ], bf16, tag="xq")
                    nc.scalar.dma_start(
                        out=xqt,
                        in_=xpb_r[
                            :, half * TBH:(half + 1) * TBH, ho:ho + NQ
                        ],
                    )
                    for k4 in range(Im // 4):
                        w2t = w2p.tile([P, 4, NQ], bf16, tag="w2")
                        nc.sync.dma_start(
                            out=w2t,
                            in_=w2_r[:, k4 * 4:(k4 + 1) * 4, ho:ho + NQ],
                        )
                        for kk in range(4):
                            k2 = k4 * 4 + kk
                            for tb in range(TBH):
                                nc.tensor.matmul(
                                    ps2[tb],
                                    lhsT=h1T[:, k2, tb * P:(tb + 1) * P],
                                    rhs=w2t[:, kk, :],
                                    start=(k2 == 0),
                                    stop=(k2 == Im - 1),
                                )
                    for tb in range(TBH):
                        j = half * TBH + tb
                        ot = outp.tile([P, NQ], f32, tag="ot")
                        nc.vector.tensor_add(
                            out=ot, in0=ps2[tb], in1=xqt[:, tb, :]
                        )
                        nc.scalar.dma_start(
                            out=out_d[j * P:(j + 1) * P, ho:ho + NQ],
                            in_=ot,
                        )


            # phase1(B) is emitted before fc2(A): its DVE/PE work hides
            # under fc1(A), and its ACT xT-copies stay ahead of fc2(A)'s
            # out-DMAs on the shared ACT queue.
            for _ in range(repeat):
                phase1(0)
                fc1(0)
                phase1(1)
                fc2(0)
                fc1(1)
                fc2(1)

    nc.compile()
    return nc


def _get_compiled(TOK=None, H=None, I=None):
    key = (TOK or _B * _S // _NCORES, H or _H, I or _I)
    if key not in _CACHE:
        _CACHE[key] = _build(*key)
    return _CACHE[key]


def _prep_weights(inter_w, inter_b, output_w, attn_nb, output_b, ln_w):
    P = 128
    H, I = inter_w.shape
    Hk, Im = H // P, I // P
    bf = ml_dtypes.bfloat16
    # h = LN(x)*ln_w + attn_nb, h1 = relu(h @ w1 + b1):
    #   fold ln_w into w1's rows and attn_nb @ w1 into b1 (exact linearity)
    w1_eff = (ln_w.astype(np.float64)[:, None] * inter_w.astype(np.float64))
    b1_eff = (
        attn_nb.astype(np.float64) @ inter_w.astype(np.float64)
        + inter_b.astype(np.float64)
    ).astype(np.float32)
    w1 = np.ascontiguousarray(
        w1_eff.astype(np.float32).reshape(Hk, P, Im, P).transpose(2, 1, 0, 3)
    ).astype(bf)
    b1 = np.ascontiguousarray(b1_eff.reshape(Im, P).T).astype(np.float32)
    w2 = np.ascontiguousarray(output_w.reshape(Im, P, H)).astype(bf)
    b2 = np.ascontiguousarray(output_b).astype(bf)
    return w1, b1, w2, b2


def _make_in_maps(inputs, n_cores=_NCORES):
    inp = np.asarray(inputs["input"], np.float32)
    res = np.asarray(inputs["residual"], np.float32)
    bias = np.asarray(inputs["bias"], np.float32)
    attn_nb = np.asarray(inputs["attn_nb"], np.float32)
    inter_w = np.asarray(inputs["inter_w"], np.float32)
    inter_b = np.asarray(inputs["inter_b"], np.float32)
    output_w = np.asarray(inputs["output_w"], np.float32)
    output_b = np.asarray(inputs["output_b"], np.float32)

    ln_w = np.asarray(inputs["ln_w"], np.float32)

    B, S, H = inp.shape
    N = B * S
    TOK = N // n_cores
    w1, b1, w2, b2 = _prep_weights(
        inter_w, inter_b, output_w, attn_nb, output_b, ln_w
    )
    bf = ml_dtypes.bfloat16
    xr = np.empty((N, 2, H), bf)
    xr[:, 0, :] = inp.reshape(N, H) + bias[None, :]
    xr[:, 1, :] = res.reshape(N, H)
    in_maps = []
    for c in range(n_cores):
        in_maps.append(
            {
                "xr_in": xr[c * TOK:(c + 1) * TOK],
                "w1": w1,
                "b1": b1,
                "w2": w2,
                "b2_row": b2,
            }
        )
    return in_maps, TOK, H, inter_w.shape[1]


def kernel(**inputs):
    # residual_norm, weight, ln_w are unused by the reference computation
    # (ln_w is all-ones).
    from concourse.bass_utils import run_bass_kernel_spmd

    in_maps, TOK, H, I = _make_in_maps(inputs)
    nc = _get_compiled(TOK, H, I)
    results = run_bass_kernel_spmd(nc, in_maps, core_ids=list(range(_NCORES)))
    out = np.concatenate(
        [results.results[c]["out"] for c in range(_NCORES)], axis=0
    )
    B, S, H = np.asarray(inputs["input"]).shape
    return out.reshape(B, S, H).astype(np.float32)

